# revision 29
# baseline (speedup 1.0000x reference)
"""Data-parallel Trainium2 attention kernel (B=8 sharded over 8 NeuronCores).

v2: software-pipelined, ACT-bound design.
Per core (one batch element):
  - qkv projections (bf16), bias via K=1 ones-matmuls, softmax scale folded
    into Wq/bq on host.
  - attention loop qt(2 q-halves, outer) x g(6 groups of 4 heads) x kt(8
    k-blocks): per kt, scores for heads 01 / 23 go into two 2-bank PSUM
    tiles via row-tiled (K=32) matmul pairs; two 1024-elem ACT exps
    (PSUM->SBUF bf16) pipeline against the next kt's score matmuls; one
    2048-elem DVE mul applies the host-precomputed exp(rel_pos) tile.
    PV + denominator col-tiled packs (pv x4 adjacent, dn x4 adjacent) lag
    one group behind the score stream so PE fills ACT's shadow.
  - qkv/v/out-proj matmul chains are emitted as "fillers" inside the kt
    loop to use remaining PE gaps.
  - normalization: denominators DMAd PSUM->DRAM (f32), repacked
    [8,512] -> [128,32] so one cheap DVE reciprocal per 2 groups, then
    partition-broadcast DMA + DVE mul on outT.
  - out projection writes PSUM->DRAM directly (f32); host adds bproj.
"""

import sys

sys.path.insert(0, "/opt/trn_rl_repo")

import numpy as np
import ml_dtypes

B, N, C, H, DH = 8, 1024, 768, 24, 32
HG = H // 4  # 6 groups of 4 heads
NQT = 2      # q halves
QW = N // NQT  # 512
SCALE = DH ** -0.5
BF16 = ml_dtypes.bfloat16

_CACHE = {}


def _build():
    if "nc" in _CACHE:
        return _CACHE["nc"]
    from contextlib import ExitStack
    import concourse.mybir as mybir
    import concourse.tile as tile
    from concourse import bacc

    nc = bacc.Bacc("TRN2")
    bf, f32 = mybir.dt.bfloat16, mybir.dt.float32
    Exp = mybir.ActivationFunctionType.Exp

    xT_d = nc.declare_dram_parameter("xT", [128, 6, N], bf, isOutput=False)
    wq_d = nc.declare_dram_parameter("wq", [128, 6, C], bf, isOutput=False)
    wk_d = nc.declare_dram_parameter("wk", [128, 6, C], bf, isOutput=False)
    wv_d = nc.declare_dram_parameter("wv", [128, 6, C], bf, isOutput=False)
    bq_d = nc.declare_dram_parameter("bq", [128, HG], f32, isOutput=False)
    bk_d = nc.declare_dram_parameter("bk", [128, HG], f32, isOutput=False)
    # eb layout: [(qt,g,kt), p(128), hl(4), q(512)]; each 512KB tile is
    # contiguous so the per-tile DMA is one big transfer.
    eb_d = nc.declare_dram_parameter(
        "expb", [NQT * HG * 8, 128, 4, QW], bf, isOutput=False)
    wp_d = nc.declare_dram_parameter("wpj", [128, 6, C], bf, isOutput=False)
    out_d = nc.declare_dram_parameter("out", [N, C], f32, isOutput=True)


    with tile.TileContext(nc) as tc, ExitStack() as ctx:
        ctx.enter_context(nc.allow_low_precision(
            reason="bf16 compute intentional; rel_err budget 2e-2"))
        const = ctx.enter_context(tc.tile_pool(name="const", bufs=1))
        big = ctx.enter_context(tc.tile_pool(name="big", bufs=1))
        stage = ctx.enter_context(tc.tile_pool(name="stage", bufs=6))
        exps_p = ctx.enter_context(tc.tile_pool(name="exps", bufs=2))
        prb = ctx.enter_context(tc.tile_pool(name="prb", bufs=12))
        norm = ctx.enter_context(tc.tile_pool(name="norm", bufs=2))
        # PSUM banks: scores 2x2 + pv 1 + dn 1 + proj/qkv 2 = 8
        psS = ctx.enter_context(tc.tile_pool(name="psS", bufs=2, space="PSUM"))
        psPV = ctx.enter_context(tc.tile_pool(name="psPV", bufs=1, space="PSUM"))
        psDN = ctx.enter_context(tc.tile_pool(name="psDN", bufs=1, space="PSUM"))
        psP = ctx.enter_context(tc.tile_pool(name="psP", bufs=2, space="PSUM"))
        scrp = ctx.enter_context(tc.tile_pool(name="scrp", bufs=3, space="DRAM"))

        # ---- constant loads (4-way partition-slab DMAs for queue overlap)
        def load_t(dram, shape, name):
            t = const.tile(shape, bf, name=name)
            for c in range(4):
                nc.sync.dma_start(
                    out=t[32 * c:32 * (c + 1)], in_=dram[32 * c:32 * (c + 1)])
            return t

        xT = load_t(xT_d, [128, 6, N], "xT")

        def load_w(dram, name):
            return load_t(dram, [128, 6, C], name)

        wq = load_w(wq_d, "wq")
        wk = load_w(wk_d, "wk")
        wv = load_w(wv_d, "wv")
        wp = load_w(wp_d, "wp")
        bq = const.tile([128, HG], f32)
        nc.sync.dma_start(out=bq, in_=bq_d[:, :])
        bk = const.tile([128, HG], f32)
        nc.sync.dma_start(out=bk, in_=bk_d[:, :])
        ones128 = const.tile([128, 1], bf)
        nc.vector.memset(ones128, 1.0)

        # ---- persistent intermediates
        qT = big.tile([128, 6, N], bf)      # q*scale+bq, [32h+d -> p], g, n
        kT = big.tile([128, 6, N], bf)
        v = big.tile([128, 8, H, DH], bf)   # [token%128, token//128, h, d]
        outT = big.tile([128, 6, N], bf)    # unnorm attn out.T [32h+d, g, n]

        # ---- chain emitters (each ~6-7 matmuls on one psP tile)
        def emit_qk_chain(gg, which, t):
            wt, bt, dstT = ((wq, bq, qT), (wk, bk, kT))[which]
            ps = psP.tile([128, 512], f32, tag="pp")
            for s in range(6):
                nc.tensor.matmul(
                    ps, lhsT=wt[:, s, 128 * gg:128 * (gg + 1)],
                    rhs=xT[:, s, 512 * t:512 * (t + 1)],
                    start=(s == 0), stop=(s == 5))
            nc.vector.tensor_scalar_add(
                out=dstT[:, gg, 512 * t:512 * (t + 1)], in0=ps,
                scalar1=bt[:, gg:gg + 1])

        def emit_v_chain(i, half):
            f0, fw = ((0, 512), (512, 256))[half]
            ps = psP.tile([128, 512], f32, tag="pp")
            pv_ = ps[:, :fw]
            for s in range(6):
                nc.tensor.matmul(
                    pv_, lhsT=xT[:, s, 128 * i:128 * (i + 1)],
                    rhs=wv[:, s, f0:f0 + fw], start=(s == 0), stop=(s == 5))
            nc.vector.tensor_copy(
                out=v[:, i, f0 // DH:(f0 + fw) // DH, :],
                in_=pv_.rearrange("p (h d) -> p h d", d=DH))

        def emit_proj_chain(j, half):
            f0, fw = ((0, 512), (512, 256))[half]
            ps = psP.tile([128, 512], f32, tag="pp")
            py = ps[:, :fw]
            for s in range(6):
                nc.tensor.matmul(
                    py, lhsT=outT[:, s, 128 * j:128 * (j + 1)],
                    rhs=wp[:, s, f0:f0 + fw], start=(s == 0), stop=(s == 5))
            yt = norm.tile([128, 512], f32, tag="yt", name="yt")
            nc.vector.tensor_copy(out=yt[:, :fw], in_=py)
            # host adds bproj
            for c in range(4):
                nc.sync.dma_start(
                    out=out_d[128 * j + 32 * c:128 * j + 32 * (c + 1),
                              f0:f0 + fw],
                    in_=yt[32 * c:32 * (c + 1), :fw])

        # filler queue of zero-arg closures, popped inside the kt loop
        fillers = []
        for which in range(2):
            for t in range(2):
                fillers.append(
                    lambda g=1, w=which, t=t: emit_qk_chain(g, w, t))
        for i in range(8):
            for half in range(2):
                fillers.append(lambda i=i, h=half: emit_v_chain(i, h))
        for gg in range(2, 6):
            for which in range(2):
                for t in range(2):
                    fillers.append(
                        lambda g=gg, w=which, t=t: emit_qk_chain(g, w, t))

        def pop_fillers(k):
            for _ in range(k):
                if fillers:
                    fillers.pop(0)()

        # ---- upfront: qk projections for group 0
        for which in range(2):
            for t in range(2):
                emit_qk_chain(0, which, t)

        def emit_eb_dma(eb_tile, qt, g, kt):
            base = (qt * HG + g) * 8 + kt
            for c in range(4):
                nc.sync.dma_start(
                    out=eb_tile[32 * c:32 * (c + 1)],
                    in_=eb_d[base, 32 * c:32 * (c + 1)])

        def emit_pv_pack(pv, dn, ptiles, g, kt):
            for hl in range(4):
                nc.tensor.matmul(
                    pv[32 * hl:32 * (hl + 1), :],
                    lhsT=v[:, kt, 4 * g + hl, :],
                    rhs=ptiles[kt][:, hl, :],
                    start=(kt == 0), stop=(kt == 7),
                    tile_position=(0, 32 * hl), skip_group_check=True)
            for hl in range(4):
                nc.tensor.matmul(
                    dn[32 * hl:32 * hl + 1, :],
                    lhsT=ones128[:, :],
                    rhs=ptiles[kt][:, hl, :],
                    start=(kt == 0), stop=(kt == 7),
                    tile_position=(0, 32 * hl), skip_group_check=True)

        sd_tiles = {}

        def emit_dn_store(qt, g, dn):
            # pull the 4 meaningful dn rows to DRAM [4,512] (4 descriptors),
            # regather as [16,128] so the DVE reciprocal is cheap, and store
            # the reciprocals back as [4,512] for the broadcast reads
            dt = norm.tile([128, QW], bf, tag="dt", name="dt")
            nc.vector.tensor_copy(out=dt, in_=dn)
            sd = scrp.tile([4, QW], bf, tag="sd", name="sd")
            nc.sync.dma_start(out=sd, in_=dt[0:128:32, :])
            rc = norm.tile([16, 128], bf, tag="rc", name="rc")
            for hl in range(4):
                nc.sync.dma_start(
                    out=rc[4 * hl:4 * (hl + 1), :],
                    in_=sd[hl:hl + 1, :]
                    .rearrange("a (p x) -> (a p) x", p=4))
            nc.vector.reciprocal(out=rc, in_=rc)
            s2 = scrp.tile([4, QW], bf, tag="s2", name="s2")
            nc.sync.dma_start(
                out=s2.rearrange("a (b x) -> (a b) x", b=4), in_=rc)
            sd_tiles[(qt, g)] = s2

        def emit_norm_muls(qt, gpair):
            qs = slice(QW * qt, QW * (qt + 1))
            for g in (gpair, gpair + 1):
                s2 = sd_tiles[(qt, g)]
                rtile = norm.tile([128, QW], bf, tag="rb", name="rtile")
                for hl in range(4):
                    nc.sync.dma_start(
                        out=rtile[32 * hl:32 * (hl + 1), :],
                        in_=s2[hl:hl + 1, :].to_broadcast((32, QW)))
                nc.vector.tensor_mul(
                    out=outT[:, g, qs], in0=outT[:, g, qs], in1=rtile)

        # persistent dn accumulator bank (memset once so unused rows are
        # initialized; accumulation groups rewrite rows {0,32,64,96})
        dnt = psDN.tile([128, QW], f32, tag="dn", name="dnt")
        nc.vector.memset(dnt, 1.0)

        # ---- main attention loop
        for qt in range(NQT):
            qs = slice(QW * qt, QW * (qt + 1))
            prev = None  # (g, ptiles, pv_tile, dn_tile)
            for g in range(HG):
                ptiles = []
                for kt in range(8):
                    eb = stage.tile([128, 4, QW], bf, tag="eb")
                    emit_eb_dma(eb, qt, g, kt)
                    sc01 = psS.tile([128, 2, QW], f32, tag="sc")
                    sc23 = psS.tile([128, 2, QW], f32, tag="sc")
                    for i in range(4):
                        sct = (sc01 if i < 2 else sc23)[:, i % 2, :]
                        nc.tensor.matmul(
                            sct,
                            lhsT=kT[32 * i:32 * (i + 1), g,
                                    128 * kt:128 * (kt + 1)],
                            rhs=qT[32 * i:32 * (i + 1), g, qs],
                            start=True, stop=True,
                            tile_position=(32 * i, 0))
                    ex = exps_p.tile([128, 4, QW], bf, tag="ex")
                    nc.scalar.activation(out=ex[:, 0:2, :], in_=sc01, func=Exp)
                    nc.scalar.activation(out=ex[:, 2:4, :], in_=sc23, func=Exp)
                    pt = prb.tile([128, 4, QW], bf, tag="probs")
                    nc.vector.tensor_mul(out=pt, in0=ex, in1=eb)
                    ptiles.append(pt)
                    # lag-1 PV for previous group
                    if prev is not None:
                        pg, pptiles, ppv, pdn = prev
                        emit_pv_pack(ppv, pdn, pptiles, pg, kt)
                        if kt == 7:
                            nc.vector.tensor_copy(
                                out=outT[:, pg, qs], in_=ppv)
                            emit_dn_store(qt, pg, pdn)
                            if pg % 2 == 1:
                                emit_norm_muls(qt, pg - 1)
                    budget = 2 if (qt == 0 and g < 2) else 1
                    pop_fillers(budget)
                npv = psPV.tile([128, QW], f32, tag="pv", name="npv")
                prev = (g, ptiles, npv, dnt)
            # drain PV for the last group of this qt half
            g, pptiles, ppv, pdn = prev
            for kt in range(8):
                emit_pv_pack(ppv, pdn, pptiles, g, kt)
            nc.vector.tensor_copy(out=outT[:, g, qs], in_=ppv)
            emit_dn_store(qt, g, pdn)
            emit_norm_muls(qt, 4)
            # out-proj chains for this half: fillers for the next half
            for j in range(4 * qt, 4 * (qt + 1)):
                for half in range(2):
                    fillers.append(
                        lambda j=j, h=half: emit_proj_chain(j, h))
        pop_fillers(len(fillers))

    nc.finalize()
    _CACHE["nc"] = nc
    return nc


def _prep_shared(shared_rel_pos, Wqkv, bqkv, Wproj, bproj):
    """Host-side weight rearrangement shared by all cores (float32 in)."""
    def dev_w(m):  # [C(contract), C(out)] -> [128, 6, C]
        return np.ascontiguousarray(m.reshape(6, 128, C).transpose(1, 0, 2))

    w3 = np.asarray(Wqkv, np.float32).reshape(H, 3, DH, C)
    wq_t = dev_w((w3[:, 0] * SCALE).transpose(2, 0, 1).reshape(C, C))
    wk_t = dev_w(w3[:, 1].transpose(2, 0, 1).reshape(C, C))
    wv_t = dev_w(w3[:, 2].transpose(2, 0, 1).reshape(C, C))
    b3 = np.asarray(bqkv, np.float32).reshape(H, 3, DH)
    bq_a = np.ascontiguousarray((b3[:, 0] * SCALE).reshape(HG, 128).T)
    bk_a = np.ascontiguousarray(b3[:, 1].reshape(HG, 128).T)
    # exp(rel): [h=(g,hl), q=(qt,qw), k=(kt,slab,p16)]
    #        -> [qt, g, kt, slab, p16, hl, qw]
    expb = np.exp(np.asarray(shared_rel_pos, np.float32))
    e = expb.reshape(HG, 4, NQT, QW, 8, 128)
    e = e.transpose(2, 0, 4, 5, 1, 3)  # qt, g, kt, p, hl, qw
    e = e.reshape(NQT * HG * 8, 128, 4, QW)
    wp_t = dev_w(np.asarray(Wproj, np.float32).T.copy())
    return {
        "wq": wq_t.astype(BF16),
        "wk": wk_t.astype(BF16),
        "wv": wv_t.astype(BF16),
        "bq": bq_a.astype(np.float32),
        "bk": bk_a.astype(np.float32),
        "expb": np.ascontiguousarray(e).astype(BF16),
        "wpj": wp_t.astype(BF16),
    }


def _in_maps(x, shared):
    x = np.asarray(x, np.float32)
    maps = []
    for b in range(B):
        m = dict(shared)
        m["xT"] = np.ascontiguousarray(
            x[b].T.reshape(6, 128, N).transpose(1, 0, 2)).astype(BF16)
        maps.append(m)
    return maps


def kernel(**inputs):
    from concourse.bass_utils import run_bass_kernel_spmd

    nc = _build()
    shared = _prep_shared(
        inputs["shared_rel_pos"], inputs["Wqkv"], inputs["bqkv"],
        inputs["Wproj"], inputs["bproj"])
    maps = _in_maps(inputs["x"], shared)
    res = run_bass_kernel_spmd(nc, maps, core_ids=list(range(B)))
    bv_vec = np.asarray(inputs["bqkv"], np.float32).reshape(H, 3, DH)[:, 2]
    bp = (np.asarray(inputs["bproj"], np.float32)
          + np.asarray(inputs["Wproj"], np.float32) @ bv_vec.reshape(C))
    out = np.stack([np.asarray(res.results[i]["out"], np.float32) + bp
                    for i in range(B)])
    return out


# revision 31
# speedup vs baseline: 1.3383x; 1.3383x over previous
"""Data-parallel Trainium2 attention kernel (B=8 sharded over 8 NeuronCores).

v2: software-pipelined, ACT-bound design.
Per core (one batch element):
  - qkv projections (bf16), bias via K=1 ones-matmuls, softmax scale folded
    into Wq/bq on host.
  - attention loop qt(2 q-halves, outer) x g(6 groups of 4 heads) x kt(8
    k-blocks): per kt, scores for heads 01 / 23 go into two 2-bank PSUM
    tiles via row-tiled (K=32) matmul pairs; two 1024-elem ACT exps
    (PSUM->SBUF bf16) pipeline against the next kt's score matmuls; one
    2048-elem DVE mul applies the host-precomputed exp(rel_pos) tile.
    PV + denominator col-tiled packs (pv x4 adjacent, dn x4 adjacent) lag
    one group behind the score stream so PE fills ACT's shadow.
  - qkv/v/out-proj matmul chains are emitted as "fillers" inside the kt
    loop to use remaining PE gaps.
  - normalization: denominators DMAd PSUM->DRAM (f32), repacked
    [8,512] -> [128,32] so one cheap DVE reciprocal per 2 groups, then
    partition-broadcast DMA + DVE mul on outT.
  - out projection writes PSUM->DRAM directly (f32); host adds bproj.
"""

import sys

sys.path.insert(0, "/opt/trn_rl_repo")

import numpy as np
import ml_dtypes

B, N, C, H, DH = 8, 1024, 768, 24, 32
HG = H // 4  # 6 groups of 4 heads
NQT = 2      # q halves
QW = N // NQT  # 512
SCALE = DH ** -0.5
BF16 = ml_dtypes.bfloat16

_CACHE = {}


def _build():
    if "nc" in _CACHE:
        return _CACHE["nc"]
    from contextlib import ExitStack
    import concourse.mybir as mybir
    import concourse.tile as tile
    from concourse import bacc

    nc = bacc.Bacc("TRN2")
    bf, f32 = mybir.dt.bfloat16, mybir.dt.float32
    Exp = mybir.ActivationFunctionType.Exp

    xT_d = nc.declare_dram_parameter("xT", [128, 6, N], bf, isOutput=False)
    wq_d = nc.declare_dram_parameter("wq", [128, 6, C], bf, isOutput=False)
    wk_d = nc.declare_dram_parameter("wk", [128, 6, C], bf, isOutput=False)
    wv_d = nc.declare_dram_parameter("wv", [128, 6, C], bf, isOutput=False)
    bq_d = nc.declare_dram_parameter("bq", [128, HG], f32, isOutput=False)
    bk_d = nc.declare_dram_parameter("bk", [128, HG], f32, isOutput=False)
    # eb layout: [(qt,g,kt), p(128), hl(4), q(512)]; each 512KB tile is
    # contiguous so the per-tile DMA is one big transfer.
    eb_d = nc.declare_dram_parameter(
        "expb", [NQT * HG * 8, 128, 4, QW], bf, isOutput=False)
    wp_d = nc.declare_dram_parameter("wpj", [128, 6, C], bf, isOutput=False)
    out_d = nc.declare_dram_parameter("out", [N, C], f32, isOutput=True)


    with tile.TileContext(nc) as tc, ExitStack() as ctx:
        ctx.enter_context(nc.allow_low_precision(
            reason="bf16 compute intentional; rel_err budget 2e-2"))
        const = ctx.enter_context(tc.tile_pool(name="const", bufs=1))
        big = ctx.enter_context(tc.tile_pool(name="big", bufs=1))
        stage = ctx.enter_context(tc.tile_pool(name="stage", bufs=8))
        exps_p = ctx.enter_context(tc.tile_pool(name="exps", bufs=2))
        prb = ctx.enter_context(tc.tile_pool(name="prb", bufs=12))
        norm = ctx.enter_context(tc.tile_pool(name="norm", bufs=2))
        # PSUM banks: scores 2x2 + pv 1 + dn 1 + proj/qkv 2 = 8
        psS = ctx.enter_context(tc.tile_pool(name="psS", bufs=2, space="PSUM"))
        psPV = ctx.enter_context(tc.tile_pool(name="psPV", bufs=1, space="PSUM"))
        psDN = ctx.enter_context(tc.tile_pool(name="psDN", bufs=1, space="PSUM"))
        psP = ctx.enter_context(tc.tile_pool(name="psP", bufs=2, space="PSUM"))
        scrp = ctx.enter_context(tc.tile_pool(name="scrp", bufs=3, space="DRAM"))

        # ---- constant loads (4-way partition-slab DMAs for queue overlap)
        def load_t(dram, shape, name):
            t = const.tile(shape, bf, name=name)
            for c in range(4):
                nc.sync.dma_start(
                    out=t[32 * c:32 * (c + 1)], in_=dram[32 * c:32 * (c + 1)])
            return t

        xT = load_t(xT_d, [128, 6, N], "xT")
        wq = load_t(wq_d, [128, 6, C], "wq")
        wk = load_t(wk_d, [128, 6, C], "wk")
        wv = const.tile([128, 6, C], bf, name="wv")
        nc.sync.dma_start(out=wv, in_=wv_d[:, :, :])
        wp = const.tile([128, 6, C], bf, name="wp")
        nc.sync.dma_start(out=wp, in_=wp_d[:, :, :])
        bq = const.tile([128, HG], f32)
        nc.sync.dma_start(out=bq, in_=bq_d[:, :])
        bk = const.tile([128, HG], f32)
        nc.sync.dma_start(out=bk, in_=bk_d[:, :])
        ones128 = const.tile([128, 1], bf)
        nc.vector.memset(ones128, 1.0)

        # ---- persistent intermediates
        qT = big.tile([128, 6, N], bf)      # q*scale+bq, [32h+d -> p], g, n
        kT = big.tile([128, 6, N], bf)
        v = big.tile([128, 8, H, DH], bf)   # [token%128, token//128, h, d]
        outT = big.tile([128, 6, N], bf)    # unnorm attn out.T [32h+d, g, n]

        # ---- chain emitters (each ~6-7 matmuls on one psP tile)
        def emit_qk_chain(gg, which, t):
            wt, bt, dstT = ((wq, bq, qT), (wk, bk, kT))[which]
            ps = psP.tile([128, 512], f32, tag="pp")
            for s in range(6):
                nc.tensor.matmul(
                    ps, lhsT=wt[:, s, 128 * gg:128 * (gg + 1)],
                    rhs=xT[:, s, 512 * t:512 * (t + 1)],
                    start=(s == 0), stop=(s == 5))
            nc.vector.tensor_scalar_add(
                out=dstT[:, gg, 512 * t:512 * (t + 1)], in0=ps,
                scalar1=bt[:, gg:gg + 1])

        def emit_v_chain(i, half):
            f0, fw = ((0, 512), (512, 256))[half]
            ps = psP.tile([128, 512], f32, tag="pp")
            pv_ = ps[:, :fw]
            for s in range(6):
                nc.tensor.matmul(
                    pv_, lhsT=xT[:, s, 128 * i:128 * (i + 1)],
                    rhs=wv[:, s, f0:f0 + fw], start=(s == 0), stop=(s == 5))
            nc.vector.tensor_copy(
                out=v[:, i, f0 // DH:(f0 + fw) // DH, :],
                in_=pv_.rearrange("p (h d) -> p h d", d=DH))

        def emit_proj_chain(j):
            yt = norm.tile([128, C], f32, tag="yt", name="yt")
            for f0, fw in ((0, 512), (512, 256)):
                ps = psP.tile([128, 512], f32, tag="pp")
                py = ps[:, :fw]
                for s in range(6):
                    nc.tensor.matmul(
                        py, lhsT=outT[:, s, 128 * j:128 * (j + 1)],
                        rhs=wp[:, s, f0:f0 + fw], start=(s == 0), stop=(s == 5))
                nc.vector.tensor_copy(out=yt[:, f0:f0 + fw], in_=py)
            # host adds bproj
            nc.sync.dma_start(out=out_d[128 * j:128 * (j + 1), :], in_=yt)

        # filler queue of zero-arg closures, popped inside the kt loop
        fillers = []
        for which in range(2):
            for t in range(2):
                fillers.append(
                    lambda g=1, w=which, t=t: emit_qk_chain(g, w, t))
        for i in range(8):
            for half in range(2):
                fillers.append(lambda i=i, h=half: emit_v_chain(i, h))
        for gg in range(2, 6):
            for which in range(2):
                for t in range(2):
                    fillers.append(
                        lambda g=gg, w=which, t=t: emit_qk_chain(g, w, t))

        def pop_fillers(k):
            for _ in range(k):
                if fillers:
                    fillers.pop(0)()

        # ---- upfront: qk projections for group 0
        for which in range(2):
            for t in range(2):
                emit_qk_chain(0, which, t)

        def emit_eb_dma(eb_tile, qt, g, kt):
            base = (qt * HG + g) * 8 + kt
            nc.sync.dma_start(out=eb_tile, in_=eb_d[base])

        def emit_pv_pack(pv, dn, ptiles, g, kt):
            for hl in range(4):
                nc.tensor.matmul(
                    pv[32 * hl:32 * (hl + 1), :],
                    lhsT=v[:, kt, 4 * g + hl, :],
                    rhs=ptiles[kt][:, hl, :],
                    start=(kt == 0), stop=(kt == 7),
                    tile_position=(0, 32 * hl), skip_group_check=True)
            for hl in range(4):
                nc.tensor.matmul(
                    dn[32 * hl:32 * hl + 1, :],
                    lhsT=ones128[:, :],
                    rhs=ptiles[kt][:, hl, :],
                    start=(kt == 0), stop=(kt == 7),
                    tile_position=(0, 32 * hl), skip_group_check=True)

        sd_tiles = {}

        def emit_dn_store(qt, g, dn):
            # pull the 4 meaningful dn rows to DRAM [4,512] (4 descriptors),
            # regather as [16,128] so the DVE reciprocal is cheap, and store
            # the reciprocals back as [4,512] for the broadcast reads
            dt = norm.tile([128, QW], bf, tag="dt", name="dt")
            nc.vector.tensor_copy(out=dt, in_=dn)
            sd = scrp.tile([4, QW], bf, tag="sd", name="sd")
            nc.gpsimd.dma_start(out=sd, in_=dt[0:128:32, :])
            rc = norm.tile([16, 128], bf, tag="rc", name="rc")
            nc.gpsimd.dma_start(
                out=rc, in_=sd.rearrange("a (p x) -> (a p) x", p=4))
            nc.vector.reciprocal(out=rc, in_=rc)
            s2 = scrp.tile([4, QW], bf, tag="s2", name="s2")
            nc.gpsimd.dma_start(
                out=s2.rearrange("a (b x) -> (a b) x", b=4), in_=rc)
            sd_tiles[(qt, g)] = s2

        def emit_norm_muls(qt, gpair):
            qs = slice(QW * qt, QW * (qt + 1))
            for g in (gpair, gpair + 1):
                s2 = sd_tiles[(qt, g)]
                rtile = norm.tile([128, QW], bf, tag="rb", name="rtile")
                for hl in range(4):
                    nc.gpsimd.dma_start(
                        out=rtile[32 * hl:32 * (hl + 1), :],
                        in_=s2[hl:hl + 1, :].to_broadcast((32, QW)))
                nc.vector.tensor_mul(
                    out=outT[:, g, qs], in0=outT[:, g, qs], in1=rtile)

        # persistent dn accumulator bank (memset once so unused rows are
        # initialized; accumulation groups rewrite rows {0,32,64,96})
        dnt = psDN.tile([128, QW], f32, tag="dn", name="dnt")
        nc.vector.memset(dnt, 1.0)

        # ---- main attention loop
        for qt in range(NQT):
            qs = slice(QW * qt, QW * (qt + 1))
            prev = None  # (g, ptiles, pv_tile, dn_tile)
            for g in range(HG):
                ptiles = []
                for kt in range(8):
                    eb = stage.tile([128, 4, QW], bf, tag="eb")
                    emit_eb_dma(eb, qt, g, kt)
                    sc01 = psS.tile([128, 2, QW], f32, tag="sc")
                    sc23 = psS.tile([128, 2, QW], f32, tag="sc")
                    for i in range(4):
                        sct = (sc01 if i < 2 else sc23)[:, i % 2, :]
                        nc.tensor.matmul(
                            sct,
                            lhsT=kT[32 * i:32 * (i + 1), g,
                                    128 * kt:128 * (kt + 1)],
                            rhs=qT[32 * i:32 * (i + 1), g, qs],
                            start=True, stop=True,
                            tile_position=(32 * i, 0))
                    ex = exps_p.tile([128, 4, QW], bf, tag="ex")
                    nc.scalar.activation(out=ex[:, 0:2, :], in_=sc01, func=Exp)
                    nc.scalar.activation(out=ex[:, 2:4, :], in_=sc23, func=Exp)
                    pt = prb.tile([128, 4, QW], bf, tag="probs")
                    nc.vector.tensor_mul(out=pt, in0=ex, in1=eb)
                    ptiles.append(pt)
                    # lag-1 PV for previous group
                    if prev is not None:
                        pg, pptiles, ppv, pdn = prev
                        emit_pv_pack(ppv, pdn, pptiles, pg, kt)
                        if kt == 7:
                            nc.vector.tensor_copy(
                                out=outT[:, pg, qs], in_=ppv)
                            emit_dn_store(qt, pg, pdn)
                            if pg % 2 == 1:
                                emit_norm_muls(qt, pg - 1)
                    budget = 2 if (qt == 0 and g < 2) else 1
                    pop_fillers(budget)
                npv = psPV.tile([128, QW], f32, tag="pv", name="npv")
                prev = (g, ptiles, npv, dnt)
            # drain PV for the last group of this qt half
            g, pptiles, ppv, pdn = prev
            for kt in range(8):
                emit_pv_pack(ppv, pdn, pptiles, g, kt)
            nc.vector.tensor_copy(out=outT[:, g, qs], in_=ppv)
            emit_dn_store(qt, g, pdn)
            emit_norm_muls(qt, 4)
            # out-proj chains for this half: fillers for the next half
            for j in range(4 * qt, 4 * (qt + 1)):
                fillers.append(lambda j=j: emit_proj_chain(j))
        pop_fillers(len(fillers))

    nc.finalize()
    _CACHE["nc"] = nc
    return nc


def _prep_shared(shared_rel_pos, Wqkv, bqkv, Wproj, bproj):
    """Host-side weight rearrangement shared by all cores (float32 in)."""
    def dev_w(m):  # [C(contract), C(out)] -> [128, 6, C]
        return np.ascontiguousarray(m.reshape(6, 128, C).transpose(1, 0, 2))

    w3 = np.asarray(Wqkv, np.float32).reshape(H, 3, DH, C)
    wq_t = dev_w((w3[:, 0] * SCALE).transpose(2, 0, 1).reshape(C, C))
    wk_t = dev_w(w3[:, 1].transpose(2, 0, 1).reshape(C, C))
    wv_t = dev_w(w3[:, 2].transpose(2, 0, 1).reshape(C, C))
    b3 = np.asarray(bqkv, np.float32).reshape(H, 3, DH)
    bq_a = np.ascontiguousarray((b3[:, 0] * SCALE).reshape(HG, 128).T)
    bk_a = np.ascontiguousarray(b3[:, 1].reshape(HG, 128).T)
    # exp(rel): [h=(g,hl), q=(qt,qw), k=(kt,slab,p16)]
    #        -> [qt, g, kt, slab, p16, hl, qw]
    expb = np.exp(np.asarray(shared_rel_pos, np.float32))
    e = expb.reshape(HG, 4, NQT, QW, 8, 128)
    e = e.transpose(2, 0, 4, 5, 1, 3)  # qt, g, kt, p, hl, qw
    e = e.reshape(NQT * HG * 8, 128, 4, QW)
    wp_t = dev_w(np.asarray(Wproj, np.float32).T.copy())
    return {
        "wq": wq_t.astype(BF16),
        "wk": wk_t.astype(BF16),
        "wv": wv_t.astype(BF16),
        "bq": bq_a.astype(np.float32),
        "bk": bk_a.astype(np.float32),
        "expb": np.ascontiguousarray(e).astype(BF16),
        "wpj": wp_t.astype(BF16),
    }


def _in_maps(x, shared):
    x = np.asarray(x, np.float32)
    maps = []
    for b in range(B):
        m = dict(shared)
        m["xT"] = np.ascontiguousarray(
            x[b].T.reshape(6, 128, N).transpose(1, 0, 2)).astype(BF16)
        maps.append(m)
    return maps


def kernel(**inputs):
    from concourse.bass_utils import run_bass_kernel_spmd

    nc = _build()
    shared = _prep_shared(
        inputs["shared_rel_pos"], inputs["Wqkv"], inputs["bqkv"],
        inputs["Wproj"], inputs["bproj"])
    maps = _in_maps(inputs["x"], shared)
    res = run_bass_kernel_spmd(nc, maps, core_ids=list(range(B)))
    bv_vec = np.asarray(inputs["bqkv"], np.float32).reshape(H, 3, DH)[:, 2]
    bp = (np.asarray(inputs["bproj"], np.float32)
          + np.asarray(inputs["Wproj"], np.float32) @ bv_vec.reshape(C))
    out = np.stack([np.asarray(res.results[i]["out"], np.float32) + bp
                    for i in range(B)])
    return out
